# revision 10
# baseline (speedup 1.0000x reference)
"""Trainium2 Bass kernel for LocalWindowAttention — single-pass version.

Computation (per batch b):
    tokens = xb[b].mean(-1)                    # [NB, C]
    Q/K/V  = tokens @ W{q,k,v} + pos           # [NB, D]
    scores = window-attn over NB (win=9, clamped) with scale 1/sqrt(D)
    ctx    = softmax(scores) @ V_window        # [NB, D]
    out    = xb[b] + (ctx @ Wo)[..., None]     # broadcast over T

Strategy: data-parallel over B across 8 NeuronCores (1 batch/core).
HBM traffic is the roofline, so xb is read ONCE (fp16, 64 MiB/core) and
held in SBUF until the residual add; the output is stored as fp16
(64 MiB/core) and upcast to fp32 on the host.  Loads ride the SP HWDGE
ring, stores the ACT HWDGE ring, so the two directions drain through
independent queues.

To make two full xb blocks + all weights fit in SBUF:
  * the four projection weights are stored as fp8 (e3m4), pre-scaled on
    the host so their values sit in e3m4's normal range; the scale
    factors are folded into pos / the softmax scale / the final
    out_tok copy-out, costing zero extra instructions
  * Q^T/K^T/V are kept per 128-row block in small sliding pools
    (bufs=2/3/3) instead of full-length tensors

The clamped gather of the reference (indices clipped at the edges,
duplicating edge rows) is reproduced exactly by adding log(multiplicity)
to the score of each unique column inside the softmax.
"""

import numpy as np
import ml_dtypes

import concourse.bass as bass
import concourse.mybir as mybir
import concourse.tile as tile
import concourse.bacc as bacc
from concourse import masks as cmasks
from concourse.bass_utils import run_bass_kernel_spmd

# Problem shapes (hardcoded per contest rules)
B, NB, C, T = 8, 1024, 1024, 32
D = 1024
WIN, HALF = 9, 4
P = 128                       # partitions
NBLK = NB // P                # 8 row blocks
CCH = C // P                  # 8 c-chunks
DCH = D // P                  # 8 d-chunks
WWIN = 192                    # window columns per block (32-aligned segs)
SCALE = 1.0 / np.sqrt(D)      # 1/32
WS = 2048.0                   # fp8 weight scale (folded: see module doc)
OSC = 1.0 / (WS * 64.0)       # out_tok descale: ctx(2048x) @ wo(64x)
SCALE_EFF = SCALE / (WS * WS)  # softmax logit scale on 2048^2-scaled scores

F32 = mybir.dt.float32
BF16 = mybir.dt.bfloat16
F16 = mybir.dt.float16
F8 = mybir.dt.float8e3

N_CORES = 8


def _w0(i):
    """Window start for block i; chosen so all V-block segments are
    32-aligned (legal matmul tile_positions)."""
    return min(max(i * P - 32, 0), NB - WWIN)


def _build_masks():
    """Per-block additive masks [NBLK, P, WWIN] (pre-divided by SCALE_EFF):
    log(multiplicity) on in-band columns (reproduces the reference's
    clamped gather exactly), -1e30 elsewhere."""
    m = np.full((NBLK, P, WWIN), -1e30, np.float32)
    for i in range(NBLK):
        w0 = _w0(i)
        for r in range(P):
            n = i * P + r
            idx = np.clip(n - HALF + np.arange(WIN), 0, NB - 1)
            u, cnt = np.unique(idx, return_counts=True)
            m[i, r, u - w0] = np.log(cnt.astype(np.float64)) / SCALE_EFF
    return m


_MASKS = _build_masks()


def _segments(i):
    """V-block segments covering window [w0, w0+WWIN) for block i as
    (blk, p0, ln, cofs): rows [p0, p0+ln) of V block `blk` correspond to
    window columns [cofs, cofs+ln). All splits are 32-aligned so both
    the attn transposes and the banded matmuls get legal tile
    positions."""
    w0 = _w0(i)
    segs = []
    lo, hi = w0, w0 + WWIN
    for blk in range(NBLK):
        b0, b1 = blk * P, (blk + 1) * P
        s0, s1 = max(lo, b0), min(hi, b1)
        if s0 < s1:
            segs.append((blk, s0 - b0, s1 - s0, s0 - w0))
    return segs


def build_nc():
    # no SWDGE DMAs in this kernel -> shrink the dynamic-DMA carveout to
    # free 8 KiB/partition of SBUF for the xb stream pool
    nc = bacc.Bacc("TRN2", target_bir_lowering=False, debug=False,
                   num_devices=N_CORES, dynamic_dma_scratch_size=8192)

    xb_h = nc.declare_dram_parameter("xbh", [NB, C, T], F16, isOutput=False)
    wq_d = nc.declare_dram_parameter("wq", [C, D], F8, isOutput=False)
    wk_d = nc.declare_dram_parameter("wk", [C, D], F8, isOutput=False)
    wv_d = nc.declare_dram_parameter("wv", [C, D], F8, isOutput=False)
    wo_d = nc.declare_dram_parameter("wo", [D, C], F8, isOutput=False)
    pos_d = nc.declare_dram_parameter("pos", [NB, D], BF16, isOutput=False)
    post_d = nc.declare_dram_parameter("post", [D, NB], BF16, isOutput=False)
    mask_d = nc.declare_dram_parameter("mask", [NBLK, P, WWIN], BF16,
                                       isOutput=False)
    out_d = nc.declare_dram_parameter("out", [NB, C, T], F16, isOutput=True)

    with tile.TileContext(nc) as tc:
        _emit(nc, tc, xb_h, wq_d, wk_d, wv_d, wo_d, pos_d, post_d,
              mask_d, out_d)

    nc.compile()
    return nc


def _emit(nc, tc, xb_h, wq_d, wk_d, wv_d, wo_d, pos_d, post_d,
          mask_d, out_d):
    from contextlib import ExitStack

    with ExitStack() as ctx:
        const_pool = ctx.enter_context(tc.tile_pool(name="const", bufs=1))
        ident = const_pool.tile([P, P], BF16)
        cmasks.make_identity(nc, ident[:])

        # weights/mask ride the ACT HWDGE ring: it is idle during the
        # pipeline fill, so these overlap the first xb loads on the SP ring
        persist = ctx.enter_context(tc.tile_pool(name="persist", bufs=1))
        mask_t = persist.tile([P, NBLK, WWIN], BF16)
        nc.scalar.dma_start(mask_t[:], mask_d.rearrange("a p w -> p a w"))

        wpool = ctx.enter_context(tc.tile_pool(name="weights", bufs=1))
        wq = wpool.tile([P, CCH, D], F8)
        wk = wpool.tile([P, CCH, D], F8)
        wv = wpool.tile([P, CCH, D], F8)
        wo = wpool.tile([P, DCH, C], F8)
        nc.scalar.dma_start(wq[:], wq_d.rearrange("(a p) d -> p a d", p=P))
        nc.scalar.dma_start(wk[:], wk_d.rearrange("(a p) d -> p a d", p=P))
        nc.scalar.dma_start(wv[:], wv_d.rearrange("(a p) d -> p a d", p=P))
        nc.scalar.dma_start(wo[:], wo_d.rearrange("(a p) c -> p a c", p=P))

        # xb stream pool: 2 full blocks + 1 tile of slack; tiles persist
        # from the phase-A load until the phase-B residual add + store
        stream = ctx.enter_context(tc.tile_pool(name="stream", bufs=17))
        ts_pool = ctx.enter_context(tc.tile_pool(name="tsum", bufs=2))
        tok_pool = ctx.enter_context(tc.tile_pool(name="tokens", bufs=1))
        tokT_pool = ctx.enter_context(tc.tile_pool(name="tokT", bufs=1))
        pos_pool = ctx.enter_context(tc.tile_pool(name="pos", bufs=1))
        qT_pool = ctx.enter_context(tc.tile_pool(name="qT", bufs=2))
        kT_pool = ctx.enter_context(tc.tile_pool(name="kT", bufs=3))
        vA_pool = ctx.enter_context(tc.tile_pool(name="vA", bufs=3))
        otok_pool = ctx.enter_context(tc.tile_pool(name="otok", bufs=2))
        att_pool = ctx.enter_context(tc.tile_pool(name="attn", bufs=1))
        ctxT_pool = ctx.enter_context(tc.tile_pool(name="ctxT", bufs=8))

        # PSUM: 8 banks total -> 4 pools x 2 bufs, shared by size class
        psTR = ctx.enter_context(
            tc.tile_pool(name="psTR", bufs=2, space="PSUM"))  # transposes
        ps512 = ctx.enter_context(
            tc.tile_pool(name="ps512", bufs=2, space="PSUM"))  # V/out_tok
        ps128 = ctx.enter_context(
            tc.tile_pool(name="ps128", bufs=2, space="PSUM"))  # QK/ctx
        psS = ctx.enter_context(
            tc.tile_pool(name="psS", bufs=2, space="PSUM"))   # scores

        xb_tiles = {}   # block -> list of 8 stream tiles (kept for phase B)
        qkv = {}        # block -> (qTt, kTt, vAt)

        def load_and_project(i):
            """Stream xb block i (kept resident), T-sum into tokens,
            transpose, project Q/K/V into per-block sliding tiles."""
            tok = tok_pool.tile([P, C], BF16, tag="tok")
            xbt = []
            for j in range(CCH):
                a = stream.tile([P, P, T], F16, tag="xb")
                nc.sync.dma_start(
                    a[:], xb_h[i * P:(i + 1) * P, j * P:(j + 1) * P, :])
                xbt.append(a)
                # bf16/fp16 tree-sum over T in a scratch tile (xb data must
                # survive for the residual); TT adds run in DVE 2x mode
                for h2 in range(2):
                    cs = slice(h2 * 64, (h2 + 1) * 64)
                    s = ts_pool.tile([P, 64, 16], F16, tag="ts")
                    nc.vector.tensor_tensor(
                        s[:], a[:, cs, 0:16], a[:, cs, 16:32],
                        op=mybir.AluOpType.add)
                    for h in (8, 4, 2):
                        nc.vector.tensor_tensor(
                            s[:, :, 0:h], s[:, :, 0:h], s[:, :, h:2 * h],
                            op=mybir.AluOpType.add)
                    with nc.allow_low_precision("tokens feed bf16 matmuls"):
                        nc.vector.tensor_reduce(
                            tok[:, j * P + h2 * 64:j * P + h2 * 64 + 64],
                            s[:, :, 0:2],
                            axis=mybir.AxisListType.X, op=mybir.AluOpType.add)
            xb_tiles[i] = xbt

            tokT = tokT_pool.tile([P, CCH, P], BF16, tag="tokT")
            for cc in range(CCH):
                pt = psTR.tile([P, P], BF16, tag="tr")
                nc.tensor.transpose(pt[:], tok[:, cc * P:(cc + 1) * P],
                                    ident[:])
                nc.scalar.copy(tokT[:, cc, :], pt[:])

            # pos is added inside the PE accumulation chain (identity
            # matmul appends pos exactly), so the PSUM copy-out runs on
            # the otherwise-idle Scalar engine instead of DVE
            ic = slice(i * P, (i + 1) * P)
            posT_t = pos_pool.tile([P, DCH, P], BF16, tag="posT")
            nc.scalar.dma_start(
                posT_t[:],
                post_d.rearrange("(a p) n -> p a n", p=P)[:, :, ic])
            qTt = qT_pool.tile([P, DCH, P], BF16, tag="qT")
            kTt = kT_pool.tile([P, DCH, P], BF16, tag="kT")
            for dd in range(DCH):
                for dst, w in ((qTt, wq), (kTt, wk)):
                    ps = ps128.tile([P, P], F32, tag="qk")
                    for cc in range(CCH):
                        nc.tensor.matmul(
                            ps[:], w[:, cc, dd * P:(dd + 1) * P],
                            tokT[:, cc, :],
                            start=(cc == 0), stop=False)
                    nc.tensor.matmul(ps[:], ident[:], posT_t[:, dd, :],
                                     start=False, stop=True)
                    nc.scalar.copy(dst[:, dd, :], ps[:])
            posn_t = pos_pool.tile([P, D], BF16, tag="posn")
            nc.scalar.dma_start(
                posn_t[:], pos_d.rearrange("(a p) d -> p a d", p=P)[:, i, :])
            vAt = vA_pool.tile([P, D], BF16, tag="vA")
            for dh in range(D // 512):
                ds_ = slice(dh * 512, (dh + 1) * 512)
                ps = ps512.tile([P, 512], F32, tag="v")
                for cc in range(CCH):
                    nc.tensor.matmul(
                        ps[:], tokT[:, cc, :], wv[:, cc, ds_],
                        start=(cc == 0), stop=False)
                nc.tensor.matmul(ps[:], ident[:], posn_t[:, ds_],
                                 start=False, stop=True)
                nc.scalar.copy(vAt[:, ds_], ps[:])
            qkv[i] = (qTt, kTt, vAt)

        def attention(i):
            """Banded attention for block i -> out_tok tile."""
            segs = _segments(i)
            qTt = qkv[i][0]

            sc = psS.tile([P, WWIN], F32, tag="sc")
            for dd in range(DCH):
                for (blk, p0, ln, cofs) in segs:
                    nc.tensor.matmul(
                        sc[:, cofs:cofs + ln], qTt[:, dd, :],
                        qkv[blk][1][:, dd, p0:p0 + ln],
                        start=(dd == 0), stop=(dd == DCH - 1))
            msk = att_pool.tile([P, WWIN], F32, tag="msk")
            nc.vector.tensor_tensor(msk[:], sc[:], mask_t[:, i, :],
                                    op=mybir.AluOpType.add)
            att = att_pool.tile([P, WWIN], F32, tag="att")
            den = att_pool.tile([P, 1], F32, tag="den")
            nc.scalar.activation(att[:], msk[:],
                                 mybir.ActivationFunctionType.Exp,
                                 scale=float(SCALE_EFF), accum_out=den[:])
            rden = att_pool.tile([P, 1], F32, tag="rden")
            nc.vector.reciprocal(rden[:], den[:])
            attb = att_pool.tile([P, WWIN], BF16, tag="attb", bufs=2)
            nc.vector.tensor_scalar_mul(attb[:], att[:], rden[:])

            # transpose attn segments (32-aligned partition placement)
            attT = []
            for (blk, p0, ln, cofs) in segs:
                pt = psTR.tile([P, P], BF16, tag="tr")
                nc.tensor.transpose(pt[p0:p0 + ln, :],
                                    attb[:, cofs:cofs + ln], ident[:],
                                    tile_position=(0, p0))
                st = att_pool.tile([P, P], BF16, tag="attT_sb", bufs=4)
                nc.scalar.copy(st[p0:p0 + ln, :], pt[p0:p0 + ln, :])
                attT.append(st)

            # ctx_T [d, n] then out_tok [n, c]
            ctxTs = []
            for dd in range(DCH):
                cps = ps128.tile([P, P], F32, tag="qk")
                for k, (blk, p0, ln, cofs) in enumerate(segs):
                    nc.tensor.matmul(
                        cps[:], qkv[blk][2][p0:p0 + ln, dd * P:(dd + 1) * P],
                        attT[k][p0:p0 + ln, :],
                        start=(k == 0), stop=(k == len(segs) - 1),
                        tile_position=(p0, 0))
                cts = ctxT_pool.tile([P, P], BF16, tag="ctxT")
                nc.scalar.copy(cts[:], cps[:])
                ctxTs.append(cts)
            otok = otok_pool.tile([P, C], F16, tag="otok")
            for ch in range(C // 512):
                cs = slice(ch * 512, (ch + 1) * 512)
                ops = ps512.tile([P, 512], F32, tag="v")
                for dd in range(DCH):
                    nc.tensor.matmul(ops[:], ctxTs[dd][:], wo[:, dd, cs],
                                     start=(dd == 0), stop=(dd == DCH - 1))
                # fold away the fp8 weight scales during PSUM copy-out
                nc.scalar.mul(otok[:, cs], ops[:], float(OSC))
            return otok

        def phase_b(i, otok):
            """In-place residual broadcast-add over T, then store.  The
            adds split across DVE (even chunks) and the idle GpSimd
            (odd chunks); stores likewise split across the ACT HWDGE
            ring and the SWDGE ring so three DMA rings run in parallel
            (loads ride the SP ring)."""
            for j, bx in enumerate(xb_tiles.pop(i)):
                ot = otok[:, j * P:(j + 1) * P]
                eng, st_eng = ((nc.vector, nc.scalar) if j % 2 == 0
                               else (nc.gpsimd, nc.scalar))
                eng.tensor_tensor(
                    bx[:], bx[:],
                    ot.unsqueeze(-1).broadcast_to((P, P, T)),
                    op=mybir.AluOpType.add)
                st_eng.dma_start(
                    out_d[i * P:(i + 1) * P, j * P:(j + 1) * P, :],
                    bx[:])

        # software pipeline: attention(i) needs Q/K/V of blocks i-1..i+1;
        # block i's xb tiles stay resident until phase_b(i) drains them,
        # and block i+2's loads recycle those slots
        load_and_project(0)
        load_and_project(1)
        for i in range(NBLK):
            otok = attention(i)
            phase_b(i, otok)
            if i + 2 < NBLK:
                load_and_project(i + 2)


_NC = None


def _get_nc():
    global _NC
    if _NC is None:
        _NC = build_nc()
    return _NC


def _prep_in_maps(xb, Wq, Wk, Wv, Wo, pos):
    bf = ml_dtypes.bfloat16
    f8 = ml_dtypes.float8_e3m4
    xb_h = np.asarray(xb, np.float32).astype(np.float16)
    # fold the 1/T of the token mean and the fp8 scale WS into the
    # projection weights; WS is unwound via SCALE_EFF / OSC on device
    wq_h = (np.asarray(Wq, np.float32) * (WS / T)).astype(f8)
    wk_h = (np.asarray(Wk, np.float32) * (WS / T)).astype(f8)
    wv_h = (np.asarray(Wv, np.float32) * (WS / T)).astype(f8)
    wo_h = (np.asarray(Wo, np.float32) * 64.0).astype(f8)
    pos_s = np.asarray(pos, np.float32) * WS
    pos_h = pos_s.astype(bf)
    post_h = np.ascontiguousarray(pos_s.T).astype(bf)
    in_maps = []
    for b in range(B):
        in_maps.append({
            "xbh": xb_h[b],
            "wq": wq_h, "wk": wk_h, "wv": wv_h, "wo": wo_h,
            "pos": pos_h, "post": post_h, "mask": _MASKS.astype(bf),
        })
    return in_maps


def kernel(xb, Wq, Wk, Wv, Wo, pos):
    nc = _get_nc()
    in_maps = _prep_in_maps(xb, Wq, Wk, Wv, Wo, pos)
    res = run_bass_kernel_spmd(nc, in_maps, core_ids=list(range(N_CORES)))
    return np.stack([np.asarray(res.results[b]["out"]).astype(np.float32)
                     for b in range(B)], axis=0)


def run_profiled(xb, Wq, Wk, Wv, Wo, pos, **kw):
    """Like kernel(), but NTFF-profiled; returns (out, BassKernelResults)."""
    import sys, types
    if "antenv.axon_hooks" not in sys.modules:
        try:
            from trn_agent_boot.trn_boot import _ntff_profile_via_ctypes
            hook = _ntff_profile_via_ctypes('/opt/axon/libaxon_pjrt.so')
            mod = types.ModuleType("antenv.axon_hooks")
            mod.get_axon_ntff_profile_hook = lambda: hook
            mod.set_axon_ntff_profile_hook = lambda h: None
            sys.modules["antenv.axon_hooks"] = mod
            import concourse.bass_utils as bu
            bu.upload_artifacts = lambda tmpdir: f"local:{tmpdir}"
        except Exception as e:
            print(f"profiling shim unavailable: {e}")
    nc = _get_nc()
    in_maps = _prep_in_maps(xb, Wq, Wk, Wv, Wo, pos)
    res = run_bass_kernel_spmd(nc, in_maps, core_ids=list(range(N_CORES)),
                               trace=True, **kw)
    out = np.stack([np.asarray(res.results[b]["out"]).astype(np.float32)
                    for b in range(B)], axis=0)
    return out, res


# revision 14
# speedup vs baseline: 1.1800x; 1.1800x over previous
"""Trainium2 Bass kernel for LocalWindowAttention — single-pass, T-major.

Computation (per batch b):
    tokens = xb[b].mean(-1)                    # [NB, C]
    Q/K/V  = tokens @ W{q,k,v} + pos           # [NB, D]
    scores = window-attn over NB (win=9, clamped) with scale 1/sqrt(D)
    ctx    = softmax(scores) @ V_window        # [NB, D]
    out    = xb[b] + (ctx @ Wo)[..., None]     # broadcast over T

Strategy: data-parallel over B across 8 NeuronCores (1 batch/core).
HBM traffic is the roofline, so xb is read ONCE (fp16, 64 MiB/core) and
held in SBUF until the residual add; the output is stored as fp16
(64 MiB/core) and upcast to fp32 on the host.  Loads ride the SP HWDGE
ring, stores the ACT HWDGE ring, so the two directions drain through
independent queues.

xb is staged host-side in T-MAJOR layout [NB, T, C] (and the output is
returned T-major, un-transposed on the host): with C innermost, every
DVE op in the kernel — the T-sum accumulation and the residual
broadcast-add (stride-0 only on the outer T dim) — keeps unit inner
stride and qualifies for the fast path, nearly halving DVE time vs the
natural [NB, C, T] layout.

SBUF fit (two full xb blocks + weights):
  * projection weights stored fp8 (e3m4), pre-scaled on the host into
    e3m4's normal range; the scales are folded into pos / the softmax
    scale / the out_tok copy-out, costing zero extra instructions
  * pos is added INSIDE the PE accumulation chain (identity matmul), so
    PSUM copy-out runs on the Scalar engine and DVE never touches it
  * Q^T/K^T/V live in small per-block sliding pools (bufs 2/3/3)

The clamped gather of the reference (indices clipped at the edges,
duplicating edge rows) is reproduced exactly by adding log(multiplicity)
to the score of each unique column inside the softmax.
"""

import numpy as np
import ml_dtypes

import concourse.bass as bass
import concourse.mybir as mybir
import concourse.tile as tile
import concourse.bacc as bacc
from concourse import masks as cmasks
from concourse.bass_utils import run_bass_kernel_spmd

# Problem shapes (hardcoded per contest rules)
B, NB, C, T = 8, 1024, 1024, 32
D = 1024
WIN, HALF = 9, 4
P = 128                       # partitions
NBLK = NB // P                # 8 row blocks
CCH = C // P                  # 8 c-chunks
DCH = D // P                  # 8 d-chunks
TCH = 4                       # T rows per stream tile (8 tiles/block)
NT = T // TCH
WWIN = 192                    # window columns per block (32-aligned segs)
SCALE = 1.0 / np.sqrt(D)      # 1/32
WS = 2048.0                   # fp8 weight scale (folded: see module doc)
OSC = 1.0 / (WS * 64.0)       # out_tok descale: ctx(2048x) @ wo(64x)
SCALE_EFF = SCALE / (WS * WS)  # softmax logit scale on 2048^2-scaled scores

F32 = mybir.dt.float32
BF16 = mybir.dt.bfloat16
F16 = mybir.dt.float16
F8 = mybir.dt.float8e3

N_CORES = 8


def _w0(i):
    """Window start for block i; chosen so all V-block segments are
    32-aligned (legal matmul tile_positions)."""
    return min(max(i * P - 32, 0), NB - WWIN)


def _build_masks():
    """Per-block additive masks [NBLK, P, WWIN] (pre-divided by SCALE_EFF):
    log(multiplicity) on in-band columns (reproduces the reference's
    clamped gather exactly), -1e30 elsewhere."""
    m = np.full((NBLK, P, WWIN), -1e30, np.float32)
    for i in range(NBLK):
        w0 = _w0(i)
        for r in range(P):
            n = i * P + r
            idx = np.clip(n - HALF + np.arange(WIN), 0, NB - 1)
            u, cnt = np.unique(idx, return_counts=True)
            m[i, r, u - w0] = np.log(cnt.astype(np.float64)) / SCALE_EFF
    return m


_MASKS = _build_masks()


def _segments(i):
    """V-block segments covering window [w0, w0+WWIN) for block i as
    (blk, p0, ln, cofs): rows [p0, p0+ln) of V block `blk` correspond to
    window columns [cofs, cofs+ln). All splits are 32-aligned so both
    the attn transposes and the banded matmuls get legal tile
    positions."""
    w0 = _w0(i)
    segs = []
    lo, hi = w0, w0 + WWIN
    for blk in range(NBLK):
        b0, b1 = blk * P, (blk + 1) * P
        s0, s1 = max(lo, b0), min(hi, b1)
        if s0 < s1:
            segs.append((blk, s0 - b0, s1 - s0, s0 - w0))
    return segs


def build_nc():
    # no SWDGE DMAs in this kernel -> shrink the dynamic-DMA carveout to
    # free 8 KiB/partition of SBUF for the xb stream pool
    nc = bacc.Bacc("TRN2", target_bir_lowering=False, debug=False,
                   num_devices=N_CORES, dynamic_dma_scratch_size=8192)

    xb_h = nc.declare_dram_parameter("xbh", [NB, T, C], F16, isOutput=False)
    wq_d = nc.declare_dram_parameter("wq", [C, D], F8, isOutput=False)
    wk_d = nc.declare_dram_parameter("wk", [C, D], F8, isOutput=False)
    wv_d = nc.declare_dram_parameter("wv", [C, D], F8, isOutput=False)
    wo_d = nc.declare_dram_parameter("wo", [D, C], F8, isOutput=False)
    pos_d = nc.declare_dram_parameter("pos", [NB, D], BF16, isOutput=False)
    post_d = nc.declare_dram_parameter("post", [D, NB], BF16, isOutput=False)
    mask_d = nc.declare_dram_parameter("mask", [NBLK, P, WWIN], BF16,
                                       isOutput=False)
    out_d = nc.declare_dram_parameter("out", [NB, T, C], F16, isOutput=True)

    with tile.TileContext(nc) as tc:
        _emit(nc, tc, xb_h, wq_d, wk_d, wv_d, wo_d, pos_d, post_d,
              mask_d, out_d)

    nc.compile()
    return nc


def _emit(nc, tc, xb_h, wq_d, wk_d, wv_d, wo_d, pos_d, post_d,
          mask_d, out_d):
    from contextlib import ExitStack

    with ExitStack() as ctx:
        const_pool = ctx.enter_context(tc.tile_pool(name="const", bufs=1))
        ident = const_pool.tile([P, P], BF16)
        cmasks.make_identity(nc, ident[:])
        # fp16 identity for the fp16 token transposes (transpose matmult
        # requires matching input dtypes)
        identF = const_pool.tile([P, P], F16)
        cmasks.make_identity(nc, identF[:])

        # weights/mask ride the ACT HWDGE ring: it is idle during the
        # pipeline fill, so these overlap the first xb loads on the SP ring
        persist = ctx.enter_context(tc.tile_pool(name="persist", bufs=1))
        mask_t = persist.tile([P, NBLK, WWIN], BF16)
        nc.scalar.dma_start(mask_t[:], mask_d.rearrange("a p w -> p a w"))

        wpool = ctx.enter_context(tc.tile_pool(name="weights", bufs=1))
        wq = wpool.tile([P, CCH, D], F8)
        wk = wpool.tile([P, CCH, D], F8)
        wv = wpool.tile([P, CCH, D], F8)
        wo = wpool.tile([P, DCH, C], F8)
        nc.scalar.dma_start(wq[:], wq_d.rearrange("(a p) d -> p a d", p=P))
        nc.scalar.dma_start(wk[:], wk_d.rearrange("(a p) d -> p a d", p=P))
        nc.scalar.dma_start(wv[:], wv_d.rearrange("(a p) d -> p a d", p=P))
        nc.scalar.dma_start(wo[:], wo_d.rearrange("(a p) c -> p a c", p=P))

        # xb stream pool: 2 full blocks + 2 tiles of slack; tiles persist
        # from the phase-A load until the phase-B residual add + store
        stream = ctx.enter_context(tc.tile_pool(name="stream", bufs=18))
        tok_pool = ctx.enter_context(tc.tile_pool(name="tokens", bufs=2))
        tokT_pool = ctx.enter_context(tc.tile_pool(name="tokT", bufs=1))
        pos_pool = ctx.enter_context(tc.tile_pool(name="pos", bufs=1))
        qT_pool = ctx.enter_context(tc.tile_pool(name="qT", bufs=2))
        kT_pool = ctx.enter_context(tc.tile_pool(name="kT", bufs=3))
        vA_pool = ctx.enter_context(tc.tile_pool(name="vA", bufs=3))
        otok_pool = ctx.enter_context(tc.tile_pool(name="otok", bufs=2))
        att_pool = ctx.enter_context(tc.tile_pool(name="attn", bufs=1))
        ctxT_pool = ctx.enter_context(tc.tile_pool(name="ctxT", bufs=8))

        # PSUM: 8 banks total -> 4 pools x 2 bufs, shared by size class
        psTR = ctx.enter_context(
            tc.tile_pool(name="psTR", bufs=2, space="PSUM"))  # transposes
        ps512 = ctx.enter_context(
            tc.tile_pool(name="ps512", bufs=2, space="PSUM"))  # V/out_tok
        ps128 = ctx.enter_context(
            tc.tile_pool(name="ps128", bufs=2, space="PSUM"))  # QK/ctx
        psS = ctx.enter_context(
            tc.tile_pool(name="psS", bufs=2, space="PSUM"))   # scores

        xb_tiles = {}   # block -> list of stream tiles (kept for phase B)
        qkv = {}        # block -> (qTt, kTt, vAt)

        def load_and_project(i, split_ring=False):
            """Stream xb block i (kept resident), accumulate the T-sum
            into tokens, transpose, project Q/K/V into sliding tiles."""
            tok = tok_pool.tile([P, C], F16, tag="tok")
            xbt = []
            for j in range(NT):
                a = stream.tile([P, TCH, C], F16, tag="xb")
                ring = nc.scalar if (split_ring and j % 2) else nc.sync
                ring.dma_start(
                    a[:], xb_h[i * P:(i + 1) * P, j * TCH:(j + 1) * TCH, :])
                xbt.append(a)
                # serial T-sum accumulation; all operands unit-inner-stride
                t0 = 2 if j == 0 else 0
                if j == 0:
                    nc.vector.tensor_tensor(
                        tok[:], a[:, 0, :], a[:, 1, :],
                        op=mybir.AluOpType.add)
                for t in range(t0, TCH):
                    nc.vector.tensor_tensor(
                        tok[:], tok[:], a[:, t, :], op=mybir.AluOpType.add)
            xb_tiles[i] = xbt

            tokT = tokT_pool.tile([P, CCH, P], F16, tag="tokT")
            for cc in range(CCH):
                pt = psTR.tile([P, P], F16, tag="tr")
                nc.tensor.transpose(pt[:], tok[:, cc * P:(cc + 1) * P],
                                    identF[:])
                nc.scalar.copy(tokT[:, cc, :], pt[:])

            # pos is added inside the PE accumulation chain (identity
            # matmul appends pos exactly), so the PSUM copy-out runs on
            # the Scalar engine and DVE never touches it
            ic = slice(i * P, (i + 1) * P)
            posT_t = pos_pool.tile([P, DCH, P], BF16, tag="posT")
            nc.scalar.dma_start(
                posT_t[:],
                post_d.rearrange("(a p) n -> p a n", p=P)[:, :, ic])
            qTt = qT_pool.tile([P, DCH, P], BF16, tag="qT")
            kTt = kT_pool.tile([P, DCH, P], BF16, tag="kT")
            for dd in range(DCH):
                for dst, w in ((qTt, wq), (kTt, wk)):
                    ps = ps128.tile([P, P], F32, tag="qk")
                    for cc in range(CCH):
                        nc.tensor.matmul(
                            ps[:], w[:, cc, dd * P:(dd + 1) * P],
                            tokT[:, cc, :],
                            start=(cc == 0), stop=False)
                    nc.tensor.matmul(ps[:], ident[:], posT_t[:, dd, :],
                                     start=False, stop=True)
                    nc.scalar.copy(dst[:, dd, :], ps[:])
            posn_t = pos_pool.tile([P, D], BF16, tag="posn")
            nc.scalar.dma_start(
                posn_t[:], pos_d.rearrange("(a p) d -> p a d", p=P)[:, i, :])
            vAt = vA_pool.tile([P, D], BF16, tag="vA")
            for dh in range(D // 512):
                ds_ = slice(dh * 512, (dh + 1) * 512)
                ps = ps512.tile([P, 512], F32, tag="v")
                for cc in range(CCH):
                    nc.tensor.matmul(
                        ps[:], tokT[:, cc, :], wv[:, cc, ds_],
                        start=(cc == 0), stop=False)
                nc.tensor.matmul(ps[:], ident[:], posn_t[:, ds_],
                                 start=False, stop=True)
                nc.scalar.copy(vAt[:, ds_], ps[:])
            qkv[i] = (qTt, kTt, vAt)

        def attention(i):
            """Banded attention for block i -> out_tok tile."""
            segs = _segments(i)
            qTt = qkv[i][0]

            sc = psS.tile([P, WWIN], F32, tag="sc")
            for dd in range(DCH):
                for (blk, p0, ln, cofs) in segs:
                    nc.tensor.matmul(
                        sc[:, cofs:cofs + ln], qTt[:, dd, :],
                        qkv[blk][1][:, dd, p0:p0 + ln],
                        start=(dd == 0), stop=(dd == DCH - 1))
            msk = att_pool.tile([P, WWIN], F32, tag="msk")
            nc.vector.tensor_tensor(msk[:], sc[:], mask_t[:, i, :],
                                    op=mybir.AluOpType.add)
            att = att_pool.tile([P, WWIN], F32, tag="att")
            den = att_pool.tile([P, 1], F32, tag="den")
            nc.scalar.activation(att[:], msk[:],
                                 mybir.ActivationFunctionType.Exp,
                                 scale=float(SCALE_EFF), accum_out=den[:])
            rden = att_pool.tile([P, 1], F32, tag="rden")
            nc.vector.reciprocal(rden[:], den[:])
            attb = att_pool.tile([P, WWIN], BF16, tag="attb", bufs=2)
            nc.vector.tensor_scalar_mul(attb[:], att[:], rden[:])

            # transpose attn segments (32-aligned partition placement)
            attT = []
            for (blk, p0, ln, cofs) in segs:
                pt = psTR.tile([P, P], BF16, tag="tr")
                nc.tensor.transpose(pt[p0:p0 + ln, :],
                                    attb[:, cofs:cofs + ln], ident[:],
                                    tile_position=(0, p0))
                st = att_pool.tile([P, P], BF16, tag="attT_sb", bufs=4)
                nc.scalar.copy(st[p0:p0 + ln, :], pt[p0:p0 + ln, :])
                attT.append(st)

            # ctx_T [d, n] then out_tok [n, c]
            ctxTs = []
            for dd in range(DCH):
                cps = ps128.tile([P, P], F32, tag="qk")
                for k, (blk, p0, ln, cofs) in enumerate(segs):
                    nc.tensor.matmul(
                        cps[:], qkv[blk][2][p0:p0 + ln, dd * P:(dd + 1) * P],
                        attT[k][p0:p0 + ln, :],
                        start=(k == 0), stop=(k == len(segs) - 1),
                        tile_position=(p0, 0))
                cts = ctxT_pool.tile([P, P], BF16, tag="ctxT")
                nc.scalar.copy(cts[:], cps[:])
                ctxTs.append(cts)
            otok = otok_pool.tile([P, C], F16, tag="otok")
            for ch in range(C // 512):
                cs = slice(ch * 512, (ch + 1) * 512)
                ops = ps512.tile([P, 512], F32, tag="v")
                for dd in range(DCH):
                    nc.tensor.matmul(ops[:], ctxTs[dd][:], wo[:, dd, cs],
                                     start=(dd == 0), stop=(dd == DCH - 1))
                # fold away the fp8 weight scales during PSUM copy-out
                nc.scalar.mul(otok[:, cs], ops[:], float(OSC))
            return otok

        def phase_b(i, otok):
            """In-place residual broadcast-add (stride-0 on the outer T
            dim keeps the DVE fast path), then store on the ACT HWDGE
            ring; the final block splits stores across both rings since
            the SP ring has no loads left."""
            for j, bx in enumerate(xb_tiles.pop(i)):
                nc.vector.tensor_tensor(
                    bx[:], bx[:],
                    otok.unsqueeze(1).broadcast_to((P, TCH, C)),
                    op=mybir.AluOpType.add)
                ring = nc.sync if (i == NBLK - 1 and j % 2) else nc.scalar
                ring.dma_start(
                    out_d[i * P:(i + 1) * P, j * TCH:(j + 1) * TCH, :],
                    bx[:])

        # software pipeline: attention(i) needs Q/K/V of blocks i-1..i+1;
        # block i's xb tiles stay resident until phase_b(i) drains them,
        # and block i+2's loads recycle those slots
        load_and_project(0)
        load_and_project(1, split_ring=True)
        for i in range(NBLK):
            otok = attention(i)
            phase_b(i, otok)
            if i + 2 < NBLK:
                load_and_project(i + 2)


_NC = None


def _get_nc():
    global _NC
    if _NC is None:
        _NC = build_nc()
    return _NC


def _prep_in_maps(xb, Wq, Wk, Wv, Wo, pos):
    bf = ml_dtypes.bfloat16
    f8 = ml_dtypes.float8_e3m4
    # T-major staging: [B, NB, C, T] -> [B, NB, T, C] fp16
    xb_h = np.ascontiguousarray(
        np.asarray(xb, np.float32).astype(np.float16).transpose(0, 1, 3, 2))
    # fold the 1/T of the token mean and the fp8 scale WS into the
    # projection weights; WS is unwound via SCALE_EFF / OSC on device
    wq_h = (np.asarray(Wq, np.float32) * (WS / T)).astype(f8)
    wk_h = (np.asarray(Wk, np.float32) * (WS / T)).astype(f8)
    wv_h = (np.asarray(Wv, np.float32) * (WS / T)).astype(f8)
    wo_h = (np.asarray(Wo, np.float32) * 64.0).astype(f8)
    pos_s = np.asarray(pos, np.float32) * WS
    pos_h = pos_s.astype(bf)
    post_h = np.ascontiguousarray(pos_s.T).astype(bf)
    in_maps = []
    for b in range(B):
        in_maps.append({
            "xbh": xb_h[b],
            "wq": wq_h, "wk": wk_h, "wv": wv_h, "wo": wo_h,
            "pos": pos_h, "post": post_h, "mask": _MASKS.astype(bf),
        })
    return in_maps


def _post(res):
    # device output is T-major fp16 [NB, T, C]; restore [B, NB, C, T] fp32
    out = np.stack([np.asarray(res.results[b]["out"]) for b in range(B)],
                   axis=0)
    return np.ascontiguousarray(out.transpose(0, 1, 3, 2)).astype(np.float32)


def kernel(xb, Wq, Wk, Wv, Wo, pos):
    nc = _get_nc()
    in_maps = _prep_in_maps(xb, Wq, Wk, Wv, Wo, pos)
    res = run_bass_kernel_spmd(nc, in_maps, core_ids=list(range(N_CORES)))
    return _post(res)


def run_profiled(xb, Wq, Wk, Wv, Wo, pos, **kw):
    """Like kernel(), but NTFF-profiled; returns (out, BassKernelResults)."""
    import sys, types
    if "antenv.axon_hooks" not in sys.modules:
        try:
            from trn_agent_boot.trn_boot import _ntff_profile_via_ctypes
            hook = _ntff_profile_via_ctypes('/opt/axon/libaxon_pjrt.so')
            mod = types.ModuleType("antenv.axon_hooks")
            mod.get_axon_ntff_profile_hook = lambda: hook
            mod.set_axon_ntff_profile_hook = lambda h: None
            sys.modules["antenv.axon_hooks"] = mod
            import concourse.bass_utils as bu
            bu.upload_artifacts = lambda tmpdir: f"local:{tmpdir}"
        except Exception as e:
            print(f"profiling shim unavailable: {e}")
    nc = _get_nc()
    in_maps = _prep_in_maps(xb, Wq, Wk, Wv, Wo, pos)
    res = run_bass_kernel_spmd(nc, in_maps, core_ids=list(range(N_CORES)),
                               trace=True, **kw)
    return _post(res), res
